# revision 1
# baseline (speedup 1.0000x reference)
"""Trainium2 Bass kernel for 16-head MHA (B=2, S=2048, D=1024, E=64).

Sharding: 8 cores = 2 batches x 4 head-groups. Each core computes 4 heads
(2 pairs of 2) for one batch and returns a partial output [2048, 1024]
(sum of its 4 heads' contributions after the output projection). Host sums
the 4 partials per batch.

Per-core pipeline (all matmuls on PE, fp32 PSUM accumulation):
  - projections QT/KT (feature-major, fp16 in, fp32r out), V (fp16 in,
    transposed on PE to token-major fp32r [V|1] tiles)
  - S^T = K Q^T per head pair, two heads row-packed in the 128x128 array
  - A^T = exp(S^T) on ACT (scale folded into W_query on host), fp32r
  - O^T accumulation with fused row-sum via the [V|1] ones column
  - softmax normalization: DVE reciprocal + GPSIMD partition-broadcast +
    DVE multiply (writes fp16 O^T)
  - output projection (fp16) accumulating both pairs, DMA out token-major
"""

import sys

sys.path.insert(0, "/opt/trn_rl_repo")

import numpy as np

import concourse.bass as bass
import concourse.bacc as bacc
import concourse.mybir as mybir
from concourse import tile
from concourse.tile_rust import add_dep_helper
from concourse.bass_interp import get_hw_module
from concourse.bass_utils import run_bass_kernel_spmd

F16 = mybir.dt.float16
F32 = mybir.dt.float32
F32R = mybir.dt.float32r
BF16 = mybir.dt.bfloat16

N_CORES = 8
T = 2048          # tokens per core (one batch)
D = 1024          # model dim
E = 64            # head dim
QC = 512          # query chunk
NQ = T // QC      # 4 query chunks
KB = 128          # key block
NKB = T // KB     # 16 key blocks
ND = D // 128     # 8 contraction chunks for projections

_CACHE = {}


def _build():
    nc = bacc.Bacc("TRN2", target_bir_lowering=False, debug=False,
                   num_devices=N_CORES)

    xqT = nc.dram_tensor("xqT", [D, T], F16, kind="ExternalInput").ap()
    xkT = nc.dram_tensor("xkT", [D, T], F16, kind="ExternalInput").ap()
    xvT = nc.dram_tensor("xvT", [D, T], F16, kind="ExternalInput").ap()
    # per-pair packed weights, layout [128, 8*128]: chunk d at cols d*128
    wq = [nc.dram_tensor(f"wq{p}", [128, D], F16, kind="ExternalInput").ap()
          for p in range(2)]
    wk = [nc.dram_tensor(f"wk{p}", [128, D], F16, kind="ExternalInput").ap()
          for p in range(2)]
    wv = [nc.dram_tensor(f"wv{p}", [128, D], F16, kind="ExternalInput").ap()
          for p in range(2)]
    wo = [nc.dram_tensor(f"wo{p}", [128, D], F16, kind="ExternalInput").ap()
          for p in range(2)]
    pout = nc.dram_tensor("pout", [T, D], F32, kind="ExternalOutput").ap()

    with tile.TileContext(nc) as tc:
        with (
            tc.tile_pool(name="consts", bufs=1) as consts,
            tc.tile_pool(name="persist", bufs=1) as persist,
            tc.tile_pool(name="xs", bufs=12) as xs,
            tc.tile_pool(name="at", bufs=3) as atp,
            tc.tile_pool(name="o2t", bufs=2) as o2tp,
            tc.tile_pool(name="os", bufs=3) as osp,
            tc.tile_pool(name="small", bufs=4) as smallp,
        ):
            # ---- constants ----
            wq_sb = [consts.tile([128, D], F16, tag=f"wq{p}", name=f"wq_sb{p}") for p in range(2)]
            wk_sb = [consts.tile([128, D], F16, tag=f"wk{p}", name=f"wk_sb{p}") for p in range(2)]
            wv_sb = [consts.tile([128, D], F16, tag=f"wv{p}", name=f"wv_sb{p}") for p in range(2)]
            wo_sb = [consts.tile([128, D], F16, tag=f"wo{p}", name=f"wo_sb{p}") for p in range(2)]
            for p in range(2):
                nc.sync.dma_start(wq_sb[p][:], wq[p][:])
                nc.sync.dma_start(wk_sb[p][:], wk[p][:])
                nc.sync.dma_start(wv_sb[p][:], wv[p][:])
                nc.sync.dma_start(wo_sb[p][:], wo[p][:])

            # ---- persistent activations ----
            # feature-major Q^T, K^T per pair: rows 0:64 head0, 64:128 head1
            qt = [[persist.tile([128, QC], F16, tag=f"qt{p}_{t}", name=f"qt{p}_{t}")
                   for t in range(NQ)] for p in range(2)]
            kt = [persist.tile([128, T], F16, tag=f"kt{p}", name=f"kt{p}") for p in range(2)]
            # token-major [V | 1] per (head, key-block): [128, 65] each
            v2 = [[persist.tile([128, 65], BF16, tag=f"v2_{h}_{b}", name=f"v2_{h}_{b}")
                   for b in range(NKB)] for h in range(4)]
            ones_f32 = consts.tile([128, 1], F32, tag="ones", name="ones_f32")
            nc.vector.memset(ones_f32[:], 1.0)
            for h in range(4):
                for b in range(NKB):
                    nc.vector.tensor_copy(v2[h][b][:, 64:65], ones_f32[:])

            # ---- phase 1: projections ----
            with (
                tc.tile_pool(name="psA", bufs=3, space="PSUM") as psA,
                tc.tile_pool(name="vtmp", bufs=1) as vtmp,
            ):
                def project(x_dram, w_sb, evac, pfx, post_p=None):
                    # d-outer so PE consumes each x chunk as its DMA lands;
                    # weights for chunk d reused across the 4 t-slices
                    xt = [None] * ND
                    for d in range(ND):
                        xt[d] = xs.tile([128, T], F16, tag="x", name=f"x{pfx}_{d}")
                        half = T // 2
                        nc.sync.dma_start(
                            xt[d][:, 0:half],
                            x_dram[d * 128:(d + 1) * 128, 0:half])
                        nc.sync.dma_start(
                            xt[d][:, half:T],
                            x_dram[d * 128:(d + 1) * 128, half:T])
                    for p in range(2):
                        pss = [psA.tile([128, QC], F32, tag="proj", name=f"proj{pfx}_{t}_{p}")
                               for t in range(NQ)]
                        for d in range(ND):
                            for t in range(NQ):
                                nc.tensor.matmul(
                                    pss[t][:], w_sb[p][:, d * 128:(d + 1) * 128],
                                    xt[d][:, t * QC:(t + 1) * QC],
                                    start=(d == 0), stop=(d == ND - 1))
                        for t in range(NQ):
                            evac(p, t, pss[t])
                        if post_p is not None:
                            post_p(p)

                def evac_kt(p, t, ps):
                    nc.scalar.activation(kt[p][:, t * QC:(t + 1) * QC],
                                         ps[:], mybir.ActivationFunctionType.Copy)

                def evac_qt(p, t, ps):
                    nc.scalar.activation(qt[p][t][:],
                                         ps[:], mybir.ActivationFunctionType.Copy)

                # K first, then V (+ transpose), then Q — lets attention start
                # as soon as possible while Q tiles still stream.
                project(xkT, wk_sb, evac_kt, "k")

                vt = [vtmp.tile([128, T], BF16, tag=f"vt{p}", name=f"vt{p}") for p in range(2)]

                def evac_vt(p, t, ps):
                    nc.vector.tensor_copy(vt[p][:, t * QC:(t + 1) * QC],
                                          ps[:])

                def transpose_v(p):
                    # token-major via DMA transpose (2-byte dtype), off the PE
                    for h in range(2):
                        for blk in range(NKB):
                            nc.sync.dma_start_transpose(
                                v2[2 * p + h][blk][:, 0:64],
                                vt[p][h * 64:(h + 1) * 64,
                                      blk * 128:(blk + 1) * 128])

                project(xvT, wv_sb, evac_vt, "v", post_p=transpose_v)

                project(xqT, wq_sb, evac_qt, "q")

            # ---- phase 2: attention + output projection ----
            with (
                tc.tile_pool(name="psS", bufs=2, space="PSUM") as psS,
                tc.tile_pool(name="psO", bufs=1, space="PSUM") as psO,
                tc.tile_pool(name="psP", bufs=2, space="PSUM") as psP,
            ):
                ost_live = {}

                def emit_outproj_group(qc, o2t, sub, oc, anchor):
                    q0 = qc * QC
                    if oc == 0:
                        ost_live[(qc, sub)] = osp.tile(
                            [128, D], F32, tag="os", name=f"os_{qc}_{sub}")
                    ost = ost_live[(qc, sub)]
                    pp = psP.tile([128, 512], F32, tag="pp", name=f"pp_{qc}_{sub}_{oc}")
                    for p in range(2):
                        mm = nc.tensor.matmul(
                            pp[:],
                            o2t[p][:, sub * 128:(sub + 1) * 128],
                            wo_sb[p][:, oc * 512:(oc + 1) * 512],
                            start=(p == 0), stop=(p == 1))
                        if p == 0 and anchor is not None:
                            add_dep_helper(mm.ins, anchor.ins, sync=False,
                                           reason="interleave outproj after S")
                    nc.vector.tensor_copy(
                        ost[:, oc * 512:(oc + 1) * 512], pp[:])
                    if oc == 1:
                        nc.sync.dma_start(
                            pout[q0 + sub * 128:q0 + (sub + 1) * 128, :],
                            ost[:])
                        del ost_live[(qc, sub)]

                def emit_outproj(qc, o2t, anchor=None):
                    for sub in range(4):
                        for oc in range(2):
                            emit_outproj_group(qc, o2t, sub, oc, anchor)

                pending = None
                for qc in range(NQ):
                    o2t = [o2tp.tile([128, QC], F16, tag=f"o2t{p}", name=f"o2t_{qc}_{p}")
                           for p in range(2)]
                    for p in range(2):
                        po = [psO.tile([65, QC], F32, tag=f"o{h}", name=f"po_{qc}_{p}_{h}")
                              for h in range(2)]
                        for kb in range(NKB):
                            k0 = kb * KB
                            ps = psS.tile([128, 2 * QC], F32, tag="s", name=f"s_{qc}_{p}_{kb}")
                            s_anchor = nc.tensor.matmul(
                                ps[:, 0:QC],
                                kt[p][0:64, k0:k0 + KB],
                                qt[p][qc][0:64, :],
                                start=True, stop=True, tile_position=(0, 0))
                            nc.tensor.matmul(
                                ps[:, QC:2 * QC],
                                kt[p][64:128, k0:k0 + KB],
                                qt[p][qc][64:128, :],
                                start=True, stop=True, tile_position=(64, 0))
                            at = atp.tile([128, 2 * QC], BF16, tag="at", name=f"at_{qc}_{p}_{kb}")
                            nc.scalar.activation(
                                at[:], ps[:], mybir.ActivationFunctionType.Exp)
                            for h in range(2):
                                nc.tensor.matmul(
                                    po[h][:],
                                    v2[2 * p + h][kb][:],
                                    at[:, h * QC:(h + 1) * QC],
                                    start=(kb == 0), stop=(kb == NKB - 1))
                            if p == 1 and pending is not None and kb % 2 == 1:
                                pqc, po2t = pending
                                emit_outproj_group(pqc, po2t, kb // 4,
                                                   (kb // 2) % 2, s_anchor)
                        # evacuate O^T fast (frees PSUM), then normalize
                        for h in range(2):
                            ot = smallp.tile([65, QC], F32, tag=f"ot{h}", name=f"ot_{qc}_{p}_{h}")
                            nc.vector.tensor_copy(ot[:], po[h][:])
                            r = smallp.tile([1, QC], F32, tag=f"r{h}", name=f"r_{qc}_{p}_{h}")
                            nc.vector.reciprocal(r[:], ot[64:65, :])
                            rb = smallp.tile([64, QC], F32, tag=f"rb{h}", name=f"rb_{qc}_{p}_{h}")
                            nc.gpsimd.partition_broadcast(rb[:], r[:])
                            nc.vector.tensor_mul(
                                o2t[p][h * 64:(h + 1) * 64, :],
                                ot[0:64, :], rb[:])
                        if p == 1:
                            pending = None
                    pending = (qc, o2t)
                emit_outproj(*pending)

    nc.compile()
    nc.m = get_hw_module(nc.m)
    return nc


def _pack_w(w_pair):
    # w_pair: [2, 1024, 64] -> [1024, 128] -> chunk-major [128, 8*128]
    w = np.concatenate([w_pair[0], w_pair[1]], axis=1)          # [1024, 128]
    return np.ascontiguousarray(
        w.reshape(ND, 128, 128).transpose(1, 0, 2).reshape(128, D))


def _pack_wo(wo_pair):
    # wo_pair: [2, 64, 1024] -> [128, 1024]
    return np.ascontiguousarray(np.concatenate([wo_pair[0], wo_pair[1]], axis=0))


def kernel(q, k, v, W_query, W_key, W_val, W_out, _trace=False):
    q = np.asarray(q, dtype=np.float32)
    k = np.asarray(k, dtype=np.float32)
    v = np.asarray(v, dtype=np.float32)
    W_query = np.asarray(W_query, dtype=np.float32)
    W_key = np.asarray(W_key, dtype=np.float32)
    W_val = np.asarray(W_val, dtype=np.float32)
    W_out = np.asarray(W_out, dtype=np.float32)

    if "nc" not in _CACHE:
        _CACHE["nc"] = _build()
    nc = _CACHE["nc"]

    norm = 1.0 / np.sqrt(E)
    xT = {}
    for b in range(2):
        xT[("q", b)] = np.ascontiguousarray(q[b].T).astype(np.float16)
        xT[("k", b)] = np.ascontiguousarray(k[b].T).astype(np.float16)
        xT[("v", b)] = np.ascontiguousarray(v[b].T).astype(np.float16)

    in_maps = []
    for c in range(N_CORES):
        b, g = c // 4, c % 4
        hs = [4 * g, 4 * g + 1, 4 * g + 2, 4 * g + 3]
        m = {
            "xqT": xT[("q", b)], "xkT": xT[("k", b)], "xvT": xT[("v", b)],
        }
        for p in range(2):
            hp = hs[2 * p:2 * p + 2]
            m[f"wq{p}"] = _pack_w(W_query[hp] * norm).astype(np.float16)
            m[f"wk{p}"] = _pack_w(W_key[hp]).astype(np.float16)
            m[f"wv{p}"] = _pack_w(W_val[hp]).astype(np.float16)
            m[f"wo{p}"] = _pack_wo(W_out[hp]).astype(np.float16)
        in_maps.append(m)

    res = run_bass_kernel_spmd(nc, in_maps, list(range(N_CORES)),
                               trace=_trace)
    parts = [res.results[c]["pout"] for c in range(N_CORES)]
    out = np.stack([
        parts[0] + parts[1] + parts[2] + parts[3],
        parts[4] + parts[5] + parts[6] + parts[7],
    ]).astype(np.float32)
    if _trace:
        _CACHE["last_result"] = res
    return out



# revision 10
# speedup vs baseline: 1.4189x; 1.4189x over previous
"""Trainium2 Bass kernel for 16-head MHA (B=2, S=2048, D=1024, E=64).

Sharding: 8 cores = 2 batches x 4 head-groups. Each core computes 4 heads
(2 pairs of 2) for one batch and returns a partial out^T [128, 8, 2048]
(fp16, feature-major). Host reassembles/transposes and sums 4 partials
per batch.

Per-core pipeline (fp16 matmuls, fp32 PSUM):
  - K/Q projections feature-major, interleaved per t-slice so attention
    can start early; evacs on DVE
  - V projection token-major on the PE (no DMA transposes), four heads
    packed [V_h|1] per key-block with fused ones columns for row sums
  - per (qc, pair): 16x [S^T (row-packed 2 heads) -> exp on ACT] streamed
    ahead, then 16x A@V in a catch-up burst (deferred-AV keeps ACT fed)
  - softmax: reciprocal_approx_fast + gpsimd broadcast + DVE multiply
    straight out of PSUM
  - output projection with wo stationary emitting out^T, fp16 DMA out
Inputs split across both HWDGE queues (sync + scalar) to halve load time.
"""

import sys

sys.path.insert(0, "/opt/trn_rl_repo")

import numpy as np

import concourse.bass as bass
import concourse.bacc as bacc
import concourse.mybir as mybir
from concourse import tile
from concourse.bass_interp import get_hw_module
from concourse.bass_utils import run_bass_kernel_spmd

F16 = mybir.dt.float16
F32 = mybir.dt.float32
BF16 = mybir.dt.bfloat16

N_CORES = 8
T = 2048          # tokens per core (one batch)
D = 1024          # model dim
E = 64            # head dim
QC = 512          # query chunk
NQ = T // QC      # 4 query chunks
KB = 128          # key block
NKB = T // KB     # 16 key blocks
ND = D // 128     # 8 contraction chunks for projections

_CACHE = {}


def _build():
    nc = bacc.Bacc("TRN2", target_bir_lowering=False, debug=False,
                   num_devices=N_CORES)

    xqT = nc.dram_tensor("xqT", [D, T], F16, kind="ExternalInput").ap()
    xkT = nc.dram_tensor("xkT", [D, T], F16, kind="ExternalInput").ap()
    xvT = nc.dram_tensor("xvT", [D, T], F16, kind="ExternalInput").ap()
    # per-pair packed weights, layout [128, 8*128]: chunk d at cols d*128
    wq = [nc.dram_tensor(f"wq{p}", [128, D], F16, kind="ExternalInput").ap()
          for p in range(2)]
    wk = [nc.dram_tensor(f"wk{p}", [128, D], F16, kind="ExternalInput").ap()
          for p in range(2)]
    # wv packed [128, 8*256]: chunk d at cols d*256, head h at +h*64
    wv = nc.dram_tensor("wv", [128, ND * 256], F16, kind="ExternalInput").ap()
    wo = [nc.dram_tensor(f"wo{p}", [128, D], F16, kind="ExternalInput").ap()
          for p in range(2)]
    # out^T partial: pout[p, dc, t] = out^T[dc*128+p, t]
    pout = nc.dram_tensor("pout", [128, ND, T], F16, kind="ExternalOutput").ap()

    with tile.TileContext(nc) as tc:
        with (
            tc.tile_pool(name="consts", bufs=1) as consts,
            tc.tile_pool(name="persist", bufs=1) as persist,
            tc.tile_pool(name="xkp", bufs=8) as xkp,
            tc.tile_pool(name="xqp", bufs=8) as xqp,
            tc.tile_pool(name="xvp", bufs=8) as xvp,
            tc.tile_pool(name="at", bufs=10) as atp,
            tc.tile_pool(name="o2t", bufs=2) as o2tp,
            tc.tile_pool(name="stage", bufs=2) as stagep,
            tc.tile_pool(name="small", bufs=2) as smallp,
            tc.tile_pool(name="psS", bufs=2, space="PSUM") as psS,
            tc.tile_pool(name="psO", bufs=1, space="PSUM") as psO,
            tc.tile_pool(name="psX", bufs=2, space="PSUM") as psX,
        ):
            # ---- weights on the scalar (ACT) HWDGE queue ----
            wq_sb = [consts.tile([128, D], F16, tag=f"wq{p}", name=f"wq_sb{p}") for p in range(2)]
            wk_sb = [consts.tile([128, D], F16, tag=f"wk{p}", name=f"wk_sb{p}") for p in range(2)]
            wo_sb = [consts.tile([128, D], F16, tag=f"wo{p}", name=f"wo_sb{p}") for p in range(2)]
            wv_sb = consts.tile([128, ND * 256], F16, tag="wv", name="wv_sb")
            for p in range(2):
                nc.scalar.dma_start(wk_sb[p][:], wk[p][:])
                nc.scalar.dma_start(wq_sb[p][:], wq[p][:])
            nc.scalar.dma_start(wv_sb[:], wv[:])
            for p in range(2):
                nc.scalar.dma_start(wo_sb[p][:], wo[p][:])

            # ---- input loads split across both HWDGE queues ----
            xk_t = [xkp.tile([128, T], F16, tag="xk", name=f"xk_{d}") for d in range(ND)]
            xq_t = [xqp.tile([128, T], F16, tag="xq", name=f"xq_{d}") for d in range(ND)]
            xv_t = [xvp.tile([128, T], F16, tag="xv", name=f"xv_{d}") for d in range(ND)]
            for d in range(ND):
                eng = nc.sync if d < 4 else nc.scalar
                eng.dma_start(xk_t[d][:], xkT[d * 128:(d + 1) * 128, :])
            for d in range(ND):
                eng = nc.sync if d < 4 else nc.scalar
                eng.dma_start(xq_t[d][:], xqT[d * 128:(d + 1) * 128, :])
            for d in range(ND):
                eng = nc.sync if d < 4 else nc.scalar
                eng.dma_start(xv_t[d][:], xvT[d * 128:(d + 1) * 128, :])

            # ---- ACT exp-table warmup (after DMA dispatches; hides the
            # ~2.7us table load inside the load phase) ----
            dwarm = consts.tile([1, 4], F32, tag="dwarm", name="dwarm")
            dwarm2 = consts.tile([1, 4], F32, tag="dwarm2", name="dwarm2")
            nc.vector.memset(dwarm[:], 0.0)
            nc.scalar.activation(dwarm2[:], dwarm[:],
                                 mybir.ActivationFunctionType.Exp)

            # ---- PE HAM warmup: junk matmuls while DMAs land ----
            psj = psX.tile([128, 512], F32, tag="x", name="junk_ps")
            for i in range(36):
                nc.tensor.matmul(psj[:, 0:128], wk_sb[0][:, 0:128],
                                 wk_sb[0][:, 0:128], start=True, stop=True)

            # ---- persistent activations ----
            kt = [persist.tile([128, T], F16, tag=f"kt{p}", name=f"kt{p}") for p in range(2)]
            qt = [[persist.tile([128, QC], F16, tag=f"qt{p}_{t}", name=f"qt{p}_{t}")
                   for t in range(NQ)] for p in range(2)]
            # token-major [Vh0|1|Vh1|1|Vh2|1|Vh3|1] per key-block: [128, 260]
            v2 = [persist.tile([128, 260], BF16, tag=f"v2_{b}", name=f"v2_{b}")
                  for b in range(NKB)]
            for b in range(NKB):
                nc.vector.memset(v2[b][:], 1.0)

            # ---- K/Q projections, interleaved per t-slice ----
            def proj_slice(x_tiles, w_sb, p, dest_ap, pfx, t):
                ps = psX.tile([128, 512], F32, tag="x", name=f"pj{pfx}_{p}_{t}")
                for d in range(ND):
                    nc.tensor.matmul(
                        ps[:], w_sb[p][:, d * 128:(d + 1) * 128],
                        x_tiles[d][:, t * QC:(t + 1) * QC],
                        start=(d == 0), stop=(d == ND - 1))
                nc.vector.tensor_copy(dest_ap, ps[:])

            for t in range(NQ):
                for p in range(2):
                    proj_slice(xk_t, wk_sb, p,
                               kt[p][:, t * QC:(t + 1) * QC], "k", t)
                for p in range(2):
                    proj_slice(xq_t, wq_sb, p, qt[p][t][:], "q", t)

            # ---- V projection token-major (emitted inside attention) ----
            def emit_vproj(tb):
                psv = psX.tile([128, 512], F32, tag="x", name=f"pv{tb}")
                for d in range(ND):
                    nc.tensor.matmul(
                        psv[:, 0:256],
                        xv_t[d][:, tb * 128:(tb + 1) * 128],
                        wv_sb[:, d * 256:(d + 1) * 256],
                        start=(d == 0), stop=(d == ND - 1))
                for h in range(4):
                    nc.vector.tensor_copy(v2[tb][:, h * 65:h * 65 + 64],
                                          psv[:, h * 64:(h + 1) * 64])

            # ---- output projection (out^T chunks, wo stationary) ----
            def emit_outproj(qc, o2t, dc):
                pp = psX.tile([128, 512], F32, tag="x", name=f"pp_{qc}_{dc}")
                for p in range(2):
                    nc.tensor.matmul(
                        pp[:], wo_sb[p][:, dc * 128:(dc + 1) * 128],
                        o2t[p][:], start=(p == 0), stop=(p == 1))
                st = stage_live[qc]
                nc.vector.tensor_copy(st[:, dc * QC:(dc + 1) * QC], pp[:])
                if dc == ND - 1:
                    nc.sync.dma_start(pout[:, :, qc * QC:(qc + 1) * QC], st[:])

            stage_live = {}
            vproj_next = [0]
            outproj_pending = []

            def interleave_work(budget_vproj, budget_outproj):
                if vproj_next[0] < NKB and budget_vproj:
                    emit_vproj(vproj_next[0])
                    vproj_next[0] += 1
                elif outproj_pending and budget_outproj:
                    pqc, po2t, dc = outproj_pending.pop(0)
                    emit_outproj(pqc, po2t, dc)

            # ---- attention ----
            for qc in range(NQ):
                o2t = [o2tp.tile([128, QC], F16, tag=f"o2t{p}", name=f"o2t_{qc}_{p}")
                       for p in range(2)]
                for p in range(2):
                    po = [psO.tile([65, QC], F32, tag=f"o{h}", name=f"po_{qc}_{p}_{h}")
                          for h in range(2)]
                    LAG = 8
                    ats = []

                    def emit_av(kb):
                        at = ats[kb]
                        for h in range(2):
                            g = 2 * p + h
                            nc.tensor.matmul(
                                po[h][:],
                                v2[kb][:, g * 65:g * 65 + 65],
                                at[:, h * QC:(h + 1) * QC],
                                start=(kb == 0), stop=(kb == NKB - 1))

                    # S^T + exp stream ahead; A@V trails by LAG blocks so the
                    # ACT exp pipeline never stalls on psO/normalization
                    for kb in range(NKB):
                        interleave_work(True, True)
                        k0 = kb * KB
                        ps = psS.tile([128, 2 * QC], F32, tag="s",
                                      name=f"s_{qc}_{p}_{kb}")
                        nc.tensor.matmul(
                            ps[:, 0:QC],
                            kt[p][0:64, k0:k0 + KB],
                            qt[p][qc][0:64, :],
                            start=True, stop=True, tile_position=(0, 0))
                        nc.tensor.matmul(
                            ps[:, QC:2 * QC],
                            kt[p][64:128, k0:k0 + KB],
                            qt[p][qc][64:128, :],
                            start=True, stop=True, tile_position=(64, 0))
                        at = atp.tile([128, 2 * QC], BF16, tag="at",
                                      name=f"at_{qc}_{p}_{kb}")
                        nc.scalar.activation(
                            at[:], ps[:], mybir.ActivationFunctionType.Exp)
                        ats.append(at)
                        if kb >= LAG:
                            emit_av(kb - LAG)
                    for kb in range(NKB - LAG, NKB):
                        emit_av(kb)
                    # softmax normalization (denominator staged to SBUF —
                    # custom-DVE recip from PSUM misreads on HW)
                    for h in range(2):
                        dsb = smallp.tile([1, QC], F32, tag=f"d{h}",
                                          name=f"d_{qc}_{p}_{h}")
                        nc.vector.tensor_copy(dsb[:], po[h][64:65, :])
                        r = smallp.tile([1, QC], F32, tag=f"r{h}",
                                        name=f"r_{qc}_{p}_{h}")
                        nc.vector.reciprocal_approx_fast(r[:], dsb[:])
                        rb = smallp.tile([64, QC], F32, tag=f"rb{h}",
                                         name=f"rb_{qc}_{p}_{h}")
                        nc.gpsimd.partition_broadcast(rb[:], r[:])
                        nc.vector.tensor_mul(
                            o2t[p][h * 64:(h + 1) * 64, :],
                            po[h][0:64, :], rb[:])
                stage_live[qc] = stagep.tile([128, ND * QC], F16, tag="stage",
                                             name=f"stage_{qc}")
                for dc in range(ND):
                    outproj_pending.append((qc, o2t, dc))
            while outproj_pending:
                pqc, po2t, dc = outproj_pending.pop(0)
                emit_outproj(pqc, po2t, dc)

    nc.compile()
    nc.m = get_hw_module(nc.m)
    return nc


def _pack_w(w_pair):
    # w_pair: [2, 1024, 64] -> [1024, 128] -> chunk-major [128, 8*128]
    w = np.concatenate([w_pair[0], w_pair[1]], axis=1)          # [1024, 128]
    return np.ascontiguousarray(
        w.reshape(ND, 128, 128).transpose(1, 0, 2).reshape(128, D))


def _pack_wv(w4):
    # w4: [4, 1024, 64] -> [128, 8*256]: wv[p, dc*256 + h*64 + e]
    return np.ascontiguousarray(
        w4.reshape(4, ND, 128, E).transpose(2, 1, 0, 3).reshape(128, ND * 256))


def _pack_wo(wo_pair):
    # wo_pair: [2, 64, 1024] -> [128, 1024]
    return np.ascontiguousarray(np.concatenate([wo_pair[0], wo_pair[1]], axis=0))


def kernel(q, k, v, W_query, W_key, W_val, W_out, _trace=False):
    q = np.asarray(q, dtype=np.float32)
    k = np.asarray(k, dtype=np.float32)
    v = np.asarray(v, dtype=np.float32)
    W_query = np.asarray(W_query, dtype=np.float32)
    W_key = np.asarray(W_key, dtype=np.float32)
    W_val = np.asarray(W_val, dtype=np.float32)
    W_out = np.asarray(W_out, dtype=np.float32)

    if "nc" not in _CACHE:
        _CACHE["nc"] = _build()
    nc = _CACHE["nc"]

    norm = 1.0 / np.sqrt(E)
    xT = {}
    for b in range(2):
        xT[("q", b)] = np.ascontiguousarray(q[b].T).astype(np.float16)
        xT[("k", b)] = np.ascontiguousarray(k[b].T).astype(np.float16)
        xT[("v", b)] = np.ascontiguousarray(v[b].T).astype(np.float16)

    in_maps = []
    for c in range(N_CORES):
        b, g = c // 4, c % 4
        hs = [4 * g, 4 * g + 1, 4 * g + 2, 4 * g + 3]
        m = {
            "xqT": xT[("q", b)], "xkT": xT[("k", b)], "xvT": xT[("v", b)],
            "wv": _pack_wv(W_val[hs]).astype(np.float16),
        }
        for p in range(2):
            hp = hs[2 * p:2 * p + 2]
            m[f"wq{p}"] = _pack_w(W_query[hp] * norm).astype(np.float16)
            m[f"wk{p}"] = _pack_w(W_key[hp]).astype(np.float16)
            m[f"wo{p}"] = _pack_wo(W_out[hp]).astype(np.float16)
        in_maps.append(m)

    res = run_bass_kernel_spmd(nc, in_maps, list(range(N_CORES)),
                               trace=_trace)
    # pout[p, dc, t] = out^T[dc*128+p, t]; out = out^T.T summed over cores
    outs = []
    for b in range(2):
        acc = np.zeros((D, T), dtype=np.float32)
        for g in range(4):
            pt = res.results[4 * b + g]["pout"].astype(np.float32)
            acc += pt.transpose(1, 0, 2).reshape(D, T)
        outs.append(acc.T)
    out = np.stack(outs).astype(np.float32)
    _CACHE["last_result"] = res
    return out
